# revision 1
# baseline (speedup 1.0000x reference)
"""Trainium2 Bass kernel for nn_CriticUAVob (attention-pool critic).

Math: for each batch item b (4096 total), two attention-pool branches over
s_b [N=128, 3], then a small MLP.  Key identity used: with P = softmax(S)
row-wise and V = s'Wv',

    mean_n (P V)[n] = (1/N) * c^T s' Wv',   c[m] = sum_n U[n,m] / Z[n]

so V is never materialized.  Per item we compute S^T = s' A~ s'^T (K=4
matmuls), U^T = exp(S^T) on ScalarE, G = U^T^T-weighted sums of s' (one
K=128 matmul whose ones-column yields Z), r = 1/Z, t = G^T r (tiny matmul),
and finally a batched MLP over all items at the end.

Sharding: pure data parallel, batch split across 8 NeuronCores.
"""
import os
import sys
import numpy as np

sys.path.insert(0, "/opt/trn_rl_repo")

import concourse.bass as bass
import concourse.tile as tile
from concourse import bacc, mybir
from concourse import bass_utils
from concourse.masks import make_identity

N_CORES = 8
B = 4096
N = 128
BC = B // N_CORES          # 512 items per core
QUADS = BC // 4            # 128 groups of 4 items
F32 = mybir.dt.float32
AF = mybir.ActivationFunctionType

_cache = {}


def _build():
    nc = bacc.Bacc(
        "TRN2",
        target_bir_lowering=False,
        debug=False,
        enable_asserts=False,
        num_devices=N_CORES,
    )
    s_t = nc.dram_tensor("s", [BC, N, 3], F32, kind="ExternalInput")
    amat_t = nc.dram_tensor("amat", [4, 8], F32, kind="ExternalInput")
    wcrs_t = nc.dram_tensor("wcrs", [4, 64], F32, kind="ExternalInput")
    wctg_t = nc.dram_tensor("wctg", [4, 64], F32, kind="ExternalInput")
    w1_t = nc.dram_tensor("w1", [64, 128], F32, kind="ExternalInput")
    w2_t = nc.dram_tensor("w2", [128, 128], F32, kind="ExternalInput")
    w3_t = nc.dram_tensor("w3", [128, 1], F32, kind="ExternalInput")
    b1_t = nc.dram_tensor("b1", [128, 1], F32, kind="ExternalInput")
    b2_t = nc.dram_tensor("b2", [128, 1], F32, kind="ExternalInput")
    b3_t = nc.dram_tensor("b3rep", [1, BC], F32, kind="ExternalInput")
    out_t = nc.dram_tensor("out", [BC, 1], F32, kind="ExternalOutput")

    s_ap = s_t.ap()

    with tile.TileContext(nc) as tc:
        with (
            tc.tile_pool(name="singles", bufs=1) as singles,
            tc.tile_pool(name="qsb", bufs=3) as qsb,
            tc.tile_pool(name="pst", bufs=4, space="PSUM") as pst,
            tc.tile_pool(name="psmall", bufs=3, space="PSUM") as psmall,
        ):
            ident = singles.tile([128, 128], F32)
            make_identity(nc, ident[:])
            amat = singles.tile([4, 8], F32)
            nc.sync.dma_start(amat[:], amat_t.ap())
            wcrs = singles.tile([4, 64], F32)
            nc.sync.dma_start(wcrs[:], wcrs_t.ap())
            wctg = singles.tile([4, 64], F32)
            nc.sync.dma_start(wctg[:], wctg_t.ap())
            w1 = singles.tile([64, 128], F32)
            nc.sync.dma_start(w1[:], w1_t.ap())
            w2 = singles.tile([128, 128], F32)
            nc.sync.dma_start(w2[:], w2_t.ap())
            w3 = singles.tile([128, 1], F32)
            nc.sync.dma_start(w3[:], w3_t.ap())
            b1 = singles.tile([128, 1], F32)
            nc.sync.dma_start(b1[:], b1_t.ap())
            b2 = singles.tile([128, 1], F32)
            nc.sync.dma_start(b2[:], b2_t.ap())
            b3r = singles.tile([1, BC], F32)
            nc.sync.dma_start(b3r[:], b3_t.ap())
            # T^T accumulator: rows k=0..3, cols = item*2 + branch
            tbig = singles.tile([4, 2 * BC], F32)

            for q in range(QUADS):
                # ---- load 4 items' s as [n, (item, k)] with a ones column
                s_nat = qsb.tile([128, 16], F32, tag="s_nat")
                src = s_ap[q * 4:(q + 1) * 4].rearrange("i n k -> n i k")
                dst = s_nat[:].rearrange("n (i f) -> n i f", i=4)
                nc.sync.dma_start(dst[:, :, 0:3], src)
                nc.gpsimd.memset(dst[:, :, 3:4], 1.0)

                # ---- transpose each item: sT[k, n] (4 rows incl ones row)
                ps_t = psmall.tile([4, 512], F32, tag="ps_sm")
                for i in range(4):
                    nc.tensor.transpose(
                        ps_t[:, i * 128:(i + 1) * 128],
                        s_nat[:, i * 4:(i + 1) * 4],
                        ident[:],
                    )
                sT = qsb.tile([4, 512], F32, tag="sT")
                nc.vector.tensor_copy(sT[:], ps_t[:])

                # ---- BT = A~ s'^T per branch (PSUM reads must be 32-aligned,
                # so two [4,512] tiles rather than one [8,512])
                ps_btr = psmall.tile([4, 512], F32, tag="ps_sm")
                ps_btt = psmall.tile([4, 512], F32, tag="ps_sm")
                nc.tensor.matmul(ps_btr[:], amat[:, 0:4], sT[:])
                nc.tensor.matmul(ps_btt[:], amat[:, 4:8], sT[:])
                bt_rs = qsb.tile([4, 512], F32, tag="bt_rs")
                bt_tg = qsb.tile([4, 512], F32, tag="bt_tg")
                nc.vector.tensor_copy(bt_rs[:], ps_btr[:])
                nc.vector.tensor_copy(bt_tg[:], ps_btt[:])

                # ---- S^T per item per branch, then exp
                st_rs = pst.tile([128, 512], F32, tag="st")
                st_tg = pst.tile([128, 512], F32, tag="st")
                for i in range(4):
                    sl = slice(i * 128, (i + 1) * 128)
                    nc.tensor.matmul(st_rs[:, sl], sT[:, sl], bt_rs[:, sl])
                    nc.tensor.matmul(st_tg[:, sl], sT[:, sl], bt_tg[:, sl])
                ut_rs = qsb.tile([128, 512], F32, tag="ut_rs")
                ut_tg = qsb.tile([128, 512], F32, tag="ut_tg")
                nc.scalar.activation(ut_rs[:], st_rs[:], AF.Exp)
                nc.scalar.activation(ut_tg[:], st_tg[:], AF.Exp)

                # ---- G = sum_m U^T[m,n] * s'[m,k]  -> [n, 4]; col 3 = Z
                ps_g = psmall.tile([128, 32], F32, tag="ps_sm")
                for i in range(4):
                    sl = slice(i * 128, (i + 1) * 128)
                    nsl = slice(i * 4, (i + 1) * 4)
                    c0 = (i * 2) * 4
                    c1 = (i * 2 + 1) * 4
                    nc.tensor.matmul(ps_g[:, c0:c0 + 4], ut_rs[:, sl], s_nat[:, nsl])
                    nc.tensor.matmul(ps_g[:, c1:c1 + 4], ut_tg[:, sl], s_nat[:, nsl])
                g_sb = qsb.tile([128, 32], F32, tag="g_sb")
                nc.vector.tensor_copy(g_sb[:], ps_g[:])
                r_sb = qsb.tile([128, 8], F32, tag="r_sb")
                g3 = g_sb[:].rearrange("n (c f) -> n c f", f=4)
                nc.vector.reciprocal(r_sb[:], g3[:, :, 3])

                # ---- t = G^T r  -> [4, 1] per (item, branch)
                ps_tt = psmall.tile([4, 8], F32, tag="ps_sm")
                for c in range(8):
                    nc.tensor.matmul(
                        ps_tt[:, c:c + 1],
                        g_sb[:, c * 4:(c + 1) * 4],
                        r_sb[:, c:c + 1],
                    )
                nc.vector.tensor_copy(tbig[:, q * 8:(q + 1) * 8], ps_tt[:])

            # ---- batched MLP over all BC items
            tb3 = tbig[:].rearrange("p (b j) -> p j b", j=2)
            ps_h = pst.tile([64, BC], F32, tag="st")
            nc.tensor.matmul(ps_h[:], wcrs[:], tb3[:, 0, :], start=True, stop=False)
            nc.tensor.matmul(ps_h[:], wctg[:], tb3[:, 1, :], start=False, stop=True)
            h_sb = singles.tile([64, BC], F32)
            nc.vector.tensor_copy(h_sb[:], ps_h[:])

            ps_z1 = pst.tile([128, BC], F32, tag="st")
            nc.tensor.matmul(ps_z1[:], w1[:], h_sb[:])
            h1 = singles.tile([128, BC], F32)
            nc.scalar.activation(h1[:], ps_z1[:], AF.Tanh, bias=b1[:])

            ps_z2 = pst.tile([128, BC], F32, tag="st")
            nc.tensor.matmul(ps_z2[:], w2[:], h1[:])
            h2 = singles.tile([128, BC], F32)
            nc.scalar.activation(h2[:], ps_z2[:], AF.Tanh, bias=b2[:])

            ps_z3 = psmall.tile([1, BC], F32, tag="ps_sm")
            nc.tensor.matmul(ps_z3[:], w3[:], h2[:])
            y_sb = singles.tile([1, BC], F32)
            nc.vector.tensor_add(y_sb[:], ps_z3[:], b3r[:])

            nc.sync.dma_start(out_t.ap().rearrange("b o -> o b"), y_sb[:])

    nc.compile()
    return nc


def _host_prep(inputs):
    f = lambda x: np.asarray(x, dtype=np.float32)
    s_obs = f(inputs["s_obs"])

    def aug(W, b):
        return np.vstack([f(W), f(b).reshape(1, -1)])  # [4, dout]

    Wq_rs = aug(inputs["Wq_rs"], inputs["bq_rs"])
    Wk_rs = aug(inputs["Wk_rs"], inputs["bk_rs"])
    Wv_rs = aug(inputs["Wv_rs"], inputs["bv_rs"])
    Wq_tg = aug(inputs["Wq_tg"], inputs["bq_tg"])
    Wk_tg = aug(inputs["Wk_tg"], inputs["bk_tg"])
    Wv_tg = aug(inputs["Wv_tg"], inputs["bv_tg"])

    scale = 1.0 / np.sqrt(16.0)
    # S^T orientation needs A~ = A^T where A = Wq' Wk'^T * scale
    At_rs = (Wq_rs @ Wk_rs.T * scale).T.astype(np.float32)
    At_tg = (Wq_tg @ Wk_tg.T * scale).T.astype(np.float32)
    amat = np.concatenate([At_rs.T, At_tg.T], axis=1).astype(np.float32)  # [4,8]

    wcrs = np.zeros((4, 64), np.float32)
    wctg = np.zeros((4, 64), np.float32)
    wcrs[:, 0:32] = Wv_rs / N
    wctg[:, 32:64] = Wv_tg / N

    w1 = f(inputs["W1"])                       # [64, 128]
    b1 = f(inputs["b1"]).reshape(128, 1)
    w2 = f(inputs["W2"])                       # [128, 128]
    b2 = f(inputs["b2"]).reshape(128, 1)
    w3 = f(inputs["W3"])                       # [128, 1]
    b3rep = np.full((1, BC), float(np.asarray(inputs["b3"]).reshape(-1)[0]),
                    np.float32)

    common = dict(amat=amat, wcrs=wcrs, wctg=wctg, w1=w1, w2=w2, w3=w3,
                  b1=b1, b2=b2, b3rep=b3rep)
    in_maps = []
    for c in range(N_CORES):
        m = dict(common)
        m["s"] = np.ascontiguousarray(s_obs[c * BC:(c + 1) * BC])
        in_maps.append(m)
    return in_maps


def kernel(**inputs):
    if "nc" not in _cache:
        _cache["nc"] = _build()
    nc = _cache["nc"]
    in_maps = _host_prep(inputs)
    trace = os.environ.get("KERNEL_TRACE", "0") == "1"
    res = bass_utils.run_bass_kernel_spmd(
        nc, in_maps, core_ids=list(range(N_CORES)), trace=trace
    )
    _cache["last"] = res
    out = np.concatenate([r["out"] for r in res.results], axis=0)
    return out.astype(np.float32)



# revision 18
# speedup vs baseline: 3.9152x; 3.9152x over previous
"""Trainium2 Bass kernel for nn_CriticUAVob (attention-pool critic).

Math per item (4096 total), per branch: S = s' A s'^T (s' = [s|1] [N,4],
A = Wq' Wk'^T / sqrt(dk)), U = exp(S), Z_n = sum_m U[n,m], pooled =
Wv'^T t / N with t_k = sum_m s'[m,k] c_m, c_m = sum_n U[n,m] / Z_n.

Layout strategy (n on partitions): 8 items per group flow through
block-diagonal "superposition" matmuls so the PE runs few, wide bf16
instructions instead of thousands of tiny fp32 ones:
  S    = P^T-chunk^T @ SZ-chunk     (4 matmuls / group, moving 512 bf16)
  U    = exp(S)                     (Scalar engine, PSUM->SBUF bf16)
  Z    = segmented row-sum of U     (DVE tensor_reduce axis=X)
  r    = 1/Z                        (DVE reciprocal)
  c    = r^T @ U                    (4 matmuls into one PSUM tile at
                                     32-aligned partition stripes)
  ct   = transpose(c)               (one XBAR DMA transpose, 4 quarters)
  t    = s_nat^T @ ct               (1 matmul; off-diagonal garbage
                                     columns are ignored at extraction)
P^T (= A^T s'^T) and the zero-padded block-diagonal SZ operand are
precomputed on the host and arrive as one 1280B-per-partition DMA per
group. Final tiny MLP is batched over all 512 items.

Sharding: pure data parallel, batch split across 8 NeuronCores.
"""
import os
import sys
import numpy as np

sys.path.insert(0, "/opt/trn_rl_repo")

import ml_dtypes
import concourse.bass as bass
import concourse.tile as tile
from concourse import bacc, mybir
from concourse import bass_utils
from concourse.masks import make_identity

N_CORES = 8
B = 4096
N = 128
BC = B // N_CORES          # 512 items per core
IPG = 8                    # items per group
NG = BC // IPG             # 64 groups
F32 = mybir.dt.float32
BF16 = mybir.dt.bfloat16
AF = mybir.ActivationFunctionType

_cache = {}


def _build():
    nc = bacc.Bacc(
        "TRN2",
        target_bir_lowering=False,
        debug=False,
        enable_asserts=False,
        num_devices=N_CORES,
    )
    szg_t = nc.dram_tensor("szg", [NG, 128, 640], BF16, kind="ExternalInput")
    sn_t = nc.dram_tensor("sn", [128, BC * 4], BF16, kind="ExternalInput")
    wcrs_t = nc.dram_tensor("wcrs", [4, 64], F32, kind="ExternalInput")
    wctg_t = nc.dram_tensor("wctg", [4, 64], F32, kind="ExternalInput")
    w1_t = nc.dram_tensor("w1", [64, 128], F32, kind="ExternalInput")
    w2_t = nc.dram_tensor("w2", [128, 128], F32, kind="ExternalInput")
    w3_t = nc.dram_tensor("w3", [128, 1], F32, kind="ExternalInput")
    b1_t = nc.dram_tensor("b1", [128, 1], F32, kind="ExternalInput")
    b2_t = nc.dram_tensor("b2", [128, 1], F32, kind="ExternalInput")
    b3_t = nc.dram_tensor("b3rep", [1, BC], F32, kind="ExternalInput")
    out_t = nc.dram_tensor("out", [BC, 1], F32, kind="ExternalOutput")

    with tile.TileContext(nc) as tc:
        with (
            tc.tile_pool(name="singles", bufs=1) as singles,
            tc.tile_pool(name="szp", bufs=3) as szp,
            tc.tile_pool(name="usb", bufs=6) as usb,
            tc.tile_pool(name="gsb", bufs=2) as gsb,
            tc.tile_pool(name="pst", bufs=3, space="PSUM") as pst,
            tc.tile_pool(name="pcs", bufs=2, space="PSUM") as pcs,
            tc.tile_pool(name="psm", bufs=1, space="PSUM") as psm,
            tc.tile_pool(name="pt4", bufs=1, space="PSUM") as pt4,
        ):
            sna = singles.tile([128, BC * 4], BF16)
            nc.sync.dma_start(sna[:], sn_t.ap())
            wcrs = singles.tile([4, 64], F32)
            nc.sync.dma_start(wcrs[:], wcrs_t.ap())
            wctg = singles.tile([4, 64], F32)
            nc.sync.dma_start(wctg[:], wctg_t.ap())
            w1 = singles.tile([64, 128], F32)
            nc.sync.dma_start(w1[:], w1_t.ap())
            w2 = singles.tile([128, 128], F32)
            nc.sync.dma_start(w2[:], w2_t.ap())
            w3 = singles.tile([128, 1], F32)
            nc.sync.dma_start(w3[:], w3_t.ap())
            b1 = singles.tile([128, 1], F32)
            nc.sync.dma_start(b1[:], b1_t.ap())
            b2 = singles.tile([128, 1], F32)
            nc.sync.dma_start(b2[:], b2_t.ap())
            b3r = singles.tile([1, BC], F32)
            nc.sync.dma_start(b3r[:], b3_t.ap())
            # t accumulator [32, (g, x, c)]: [4j+k, 16g+4x+c] holds t of
            # item 8g+2c+(x//2), branch x%2, when j == 2c + x//2
            tball = singles.tile([32, NG * 16], F32)
            ident32 = singles.tile([32, 32], F32)
            make_identity(nc, ident32[:])

            for g in range(NG):
                szg = szp.tile([128, 640], BF16, tag="sz")
                nc.sync.dma_start(szg[:], szg_t.ap()[g])

                zs = gsb.tile([128, 16], BF16, tag="zs")
                rr = gsb.tile([128, 16], BF16, tag="rr")
                cps = pcs.tile([128, 512], F32, tag="c")

                for c in range(4):
                    # S chunk: [128 n, (di, b, m) 512]
                    sp = pst.tile([128, 512], F32, tag="s")
                    nc.tensor.matmul(
                        sp[:],
                        szg[32 * c:32 * c + 16, 512:640],
                        szg[32 * c:32 * c + 16, 0:512],
                        tile_position=(32 * c, 0),
                    )
                    u = usb.tile([128, 512], BF16, tag="u")
                    nc.scalar.activation(u[:], sp[:], AF.Exp)
                    u3 = u[:].rearrange("p (s m) -> p s m", s=4)
                    with nc.allow_low_precision("softmax Z in bf16"):
                        nc.vector.tensor_reduce(
                            zs[:, 4 * c:4 * c + 4], u3,
                            mybir.AxisListType.X, mybir.AluOpType.add,
                        )
                        nc.vector.reciprocal(
                            rr[:, 4 * c:4 * c + 4], zs[:, 4 * c:4 * c + 4]
                        )
                    # c rows land on the 32c partition stripe; valid columns
                    # are this chunk's quarter, rest is ignored garbage
                    nc.tensor.matmul(
                        cps[32 * c:32 * c + 4, :],
                        rr[:, 4 * c:4 * c + 4],
                        u[:],
                        tile_position=(0, 32 * c),
                    )

                cs = gsb.tile([128, 512], BF16, tag="cs")
                nc.vector.tensor_copy(cs[:], cps[:])
                # XBAR transpose: ct[m, 128q + p] = cs[p, 128q + m]
                ct = gsb.tile([128, 512], BF16, tag="ct")
                nc.sync.dma_start_transpose(
                    ct[:].rearrange("p (q f) -> p q f", q=4), cs[:]
                )

                # t: [32 (j,k), 512 (q,p)]; valid at p=32(j//2)+x, q=x,
                # x=2(j%2)+b; garbage columns dropped by the strided copies
                tp = psm.tile([32, 512], F32, tag="t")
                nc.tensor.matmul(
                    tp[:],
                    sna[:, g * 32:(g + 1) * 32],
                    ct[:],
                )
                # valid col for (x, c) is 129x+32c; extract [32, 4] per x
                tp3 = tp[:].rearrange("p (q f) -> p q f", q=4)
                tbv = tball[:].rearrange("p (g x c) -> p g x c", g=NG, x=4)
                for x in range(4):
                    nc.vector.tensor_copy(
                        tbv[:, g, x, :],
                        tp3[:, x, :].rearrange("p (c r) -> p c r", c=4)[:, :, x],
                    )

            # ---- row-gather t entries via selection matmuls:
            # tb4ps[k, (j, g, b)] = tball[4j+k, 16g + 8*(j%2) + 4b + j//2]
            tb4ps = pt4.tile([4, 1024], F32, tag="t4")
            tbx = tball[:].rearrange("p (g x2 b c) -> p g x2 b c", g=NG, x2=2, b=2)
            for j in range(8):
                nc.tensor.matmul(
                    tb4ps[:, j * 128:(j + 1) * 128],
                    ident32[:, 4 * j:4 * j + 4],
                    tbx[:, :, j % 2, :, j // 2],
                )
            # reorder (j, g, b) -> (g, j, b): tb4 [4, 2*BC]
            tb4 = singles.tile([4, 2 * BC], F32)
            nc.vector.tensor_copy(
                tb4[:].rearrange("k (g j b) -> k j g b", g=NG, j=8),
                tb4ps[:].rearrange("k (j g b) -> k j g b", g=NG, j=8),
            )

            # ---- batched MLP over all BC items
            tb3 = tb4[:].rearrange("p (b j) -> p j b", j=2)
            ps_h = pst.tile([64, BC], F32, tag="s")
            nc.tensor.matmul(ps_h[:], wcrs[:], tb3[:, 0, :], start=True, stop=False)
            nc.tensor.matmul(ps_h[:], wctg[:], tb3[:, 1, :], start=False, stop=True)
            h_sb = singles.tile([64, BC], F32)
            nc.vector.tensor_copy(h_sb[:], ps_h[:])

            ps_z1 = pst.tile([128, BC], F32, tag="s")
            nc.tensor.matmul(ps_z1[:], w1[:], h_sb[:])
            h1 = singles.tile([128, BC], F32)
            nc.scalar.activation(h1[:], ps_z1[:], AF.Tanh, bias=b1[:])

            ps_z2 = pst.tile([128, BC], F32, tag="s")
            nc.tensor.matmul(ps_z2[:], w2[:], h1[:])
            h2 = singles.tile([128, BC], F32)
            nc.scalar.activation(h2[:], ps_z2[:], AF.Tanh, bias=b2[:])

            ps_z3 = psm.tile([1, BC], F32, tag="t")
            nc.tensor.matmul(ps_z3[:], w3[:], h2[:])
            y_sb = singles.tile([1, BC], F32)
            nc.vector.tensor_add(y_sb[:], ps_z3[:], b3r[:])

            nc.sync.dma_start(out_t.ap().rearrange("b o -> o b"), y_sb[:])

    nc.compile()
    return nc


def _host_prep(inputs):
    f = lambda x: np.asarray(x, dtype=np.float32)
    s_obs = f(inputs["s_obs"])

    def aug(W, b):
        return np.vstack([f(W), f(b).reshape(1, -1)])  # [4, dout]

    Wq_rs = aug(inputs["Wq_rs"], inputs["bq_rs"])
    Wk_rs = aug(inputs["Wk_rs"], inputs["bk_rs"])
    Wv_rs = aug(inputs["Wv_rs"], inputs["bv_rs"])
    Wq_tg = aug(inputs["Wq_tg"], inputs["bq_tg"])
    Wk_tg = aug(inputs["Wk_tg"], inputs["bk_tg"])
    Wv_tg = aug(inputs["Wv_tg"], inputs["bv_tg"])

    scale = 1.0 / np.sqrt(16.0)
    A_rs = (Wq_rs @ Wk_rs.T * scale).astype(np.float32)  # [4, 4]
    A_tg = (Wq_tg @ Wk_tg.T * scale).astype(np.float32)

    wcrs = np.zeros((4, 64), np.float32)
    wctg = np.zeros((4, 64), np.float32)
    wcrs[:, 0:32] = Wv_rs / N
    wctg[:, 32:64] = Wv_tg / N

    w1 = f(inputs["W1"])
    b1 = f(inputs["b1"]).reshape(128, 1)
    w2 = f(inputs["W2"])
    b2 = f(inputs["b2"]).reshape(128, 1)
    w3 = f(inputs["W3"])
    b3rep = np.full((1, BC), float(np.asarray(inputs["b3"]).reshape(-1)[0]),
                    np.float32)

    common = dict(wcrs=wcrs, wctg=wctg, w1=w1, w2=w2, w3=w3,
                  b1=b1, b2=b2, b3rep=b3rep)
    in_maps = []
    for core in range(N_CORES):
        sc = s_obs[core * BC:(core + 1) * BC]               # [512, 128, 3]
        sa = np.concatenate(
            [sc, np.ones((BC, N, 1), np.float32)], axis=2)  # [512, 128, 4]
        sa_bf = sa.astype(ml_dtypes.bfloat16)

        # sn [128, 512*4]: [n, 4i+k] = s'_i[n, k]
        sn = np.ascontiguousarray(
            sa_bf.transpose(1, 0, 2).reshape(128, BC * 4))

        # szg [NG, 128, 640]:
        #   cols 0..511:  [32c+8di+4b+k', 256di+128b+m] = s'_{8g+2c+di}[m,k']
        #   cols 512..639: [32c+8di+4b+k', 512+n] = P_{8g+2c+di, b}[n, k']
        szg = np.zeros((NG, 128, 640), ml_dtypes.bfloat16)
        blk = sa_bf.transpose(0, 2, 1).reshape(NG, 8, 4, 128)  # [g, j, k, m]
        Ps = [
            np.einsum('ink,kq->inq', sa, A).astype(ml_dtypes.bfloat16)
            .reshape(NG, 8, 128, 4)
            for A in (A_rs, A_tg)
        ]
        for j in range(8):
            r0 = 32 * (j // 2) + 8 * (j % 2)
            c0 = 256 * (j % 2)
            for b in range(2):
                szg[:, r0 + 4 * b:r0 + 4 * b + 4,
                    c0 + 128 * b:c0 + 128 * b + 128] = blk[:, j]
                szg[:, r0 + 4 * b:r0 + 4 * b + 4, 512:640] = \
                    Ps[b][:, j].transpose(0, 2, 1)

        m = dict(common)
        m["szg"] = szg
        m["sn"] = sn
        in_maps.append(m)
    return in_maps


def kernel(**inputs):
    if "nc" not in _cache:
        _cache["nc"] = _build()
    nc = _cache["nc"]
    in_maps = _host_prep(inputs)
    trace = os.environ.get("KERNEL_TRACE", "0") == "1"
    res = bass_utils.run_bass_kernel_spmd(
        nc, in_maps, core_ids=list(range(N_CORES)), trace=trace
    )
    _cache["last"] = res
    out = np.concatenate([r["out"] for r in res.results], axis=0)
    return out.astype(np.float32)


# revision 19
# speedup vs baseline: 4.9334x; 1.2601x over previous
"""Trainium2 Bass kernel for nn_CriticUAVob (attention-pool critic).

Math per item (4096 total), per branch: S = s' A s'^T (s' = [s|1] [N,4],
A = Wq' Wk'^T / sqrt(dk)), U = exp(S), Z_n = sum_m U[n,m], pooled =
Wv'^T t / N with t_k = sum_m s'[m,k] c_m, c_m = sum_n U[n,m] / Z_n.

Layout strategy (n on partitions): 8 items per group flow through
block-diagonal "superposition" matmuls so the PE runs few, wide bf16
instructions instead of thousands of tiny fp32 ones:
  S    = P^T-chunk^T @ SZ-chunk     (4 matmuls / group, moving 512 bf16)
  U    = exp(S)                     (Scalar engine, PSUM->SBUF bf16)
  Z    = segmented row-sum of U     (DVE tensor_reduce axis=X)
  r    = 1/Z                        (DVE reciprocal)
  c    = r^T @ U                    (4 matmuls into one PSUM tile at
                                     32-aligned partition stripes)
  ct   = transpose(c)               (one XBAR DMA transpose, 4 quarters)
  t    = s_nat^T @ ct               (1 matmul; off-diagonal garbage
                                     columns are ignored at extraction)
P^T (= A^T s'^T) and the zero-padded block-diagonal SZ operand are
precomputed on the host and arrive as one 1280B-per-partition DMA per
group. Final tiny MLP is batched over all 512 items.

Sharding: pure data parallel, batch split across 8 NeuronCores.
"""
import os
import sys
import numpy as np

sys.path.insert(0, "/opt/trn_rl_repo")

import ml_dtypes
import concourse.bass as bass
import concourse.tile as tile
from concourse import bacc, mybir
from concourse import bass_utils
from concourse.masks import make_identity

N_CORES = 8
B = 4096
N = 128
BC = B // N_CORES          # 512 items per core
IPG = 8                    # items per group
NG = BC // IPG             # 64 groups
F32 = mybir.dt.float32
BF16 = mybir.dt.bfloat16
AF = mybir.ActivationFunctionType

_cache = {}


def _build():
    nc = bacc.Bacc(
        "TRN2",
        target_bir_lowering=False,
        debug=False,
        enable_asserts=False,
        num_devices=N_CORES,
    )
    szg_t = nc.dram_tensor("szg", [NG, 128, 640], BF16, kind="ExternalInput")
    sn_t = nc.dram_tensor("sn", [128, BC * 4], BF16, kind="ExternalInput")
    wcrs_t = nc.dram_tensor("wcrs", [4, 64], F32, kind="ExternalInput")
    wctg_t = nc.dram_tensor("wctg", [4, 64], F32, kind="ExternalInput")
    w1_t = nc.dram_tensor("w1", [64, 128], F32, kind="ExternalInput")
    w2_t = nc.dram_tensor("w2", [128, 128], F32, kind="ExternalInput")
    w3_t = nc.dram_tensor("w3", [128, 1], F32, kind="ExternalInput")
    b1_t = nc.dram_tensor("b1", [128, 1], F32, kind="ExternalInput")
    b2_t = nc.dram_tensor("b2", [128, 1], F32, kind="ExternalInput")
    b3_t = nc.dram_tensor("b3rep", [1, BC], F32, kind="ExternalInput")
    out_t = nc.dram_tensor("out", [BC, 1], F32, kind="ExternalOutput")

    with tile.TileContext(nc) as tc:
        with (
            tc.tile_pool(name="singles", bufs=1) as singles,
            tc.tile_pool(name="szp", bufs=3) as szp,
            tc.tile_pool(name="usb", bufs=6) as usb,
            tc.tile_pool(name="gsb", bufs=2) as gsb,
            tc.tile_pool(name="pst", bufs=3, space="PSUM") as pst,
            tc.tile_pool(name="pcs", bufs=2, space="PSUM") as pcs,
            tc.tile_pool(name="psm", bufs=1, space="PSUM") as psm,
            tc.tile_pool(name="pt4", bufs=1, space="PSUM") as pt4,
        ):
            sna = singles.tile([128, BC * 4], BF16)
            nc.sync.dma_start(sna[:], sn_t.ap())
            wcrs = singles.tile([4, 64], F32)
            nc.sync.dma_start(wcrs[:], wcrs_t.ap())
            wctg = singles.tile([4, 64], F32)
            nc.sync.dma_start(wctg[:], wctg_t.ap())
            w1 = singles.tile([64, 128], F32)
            nc.sync.dma_start(w1[:], w1_t.ap())
            w2 = singles.tile([128, 128], F32)
            nc.sync.dma_start(w2[:], w2_t.ap())
            w3 = singles.tile([128, 1], F32)
            nc.sync.dma_start(w3[:], w3_t.ap())
            b1 = singles.tile([128, 1], F32)
            nc.sync.dma_start(b1[:], b1_t.ap())
            b2 = singles.tile([128, 1], F32)
            nc.sync.dma_start(b2[:], b2_t.ap())
            b3r = singles.tile([1, BC], F32)
            nc.sync.dma_start(b3r[:], b3_t.ap())
            # t accumulator [32, (g, x, c)]: [4j+k, 16g+4x+c] holds t of
            # item 8g+2c+(x//2), branch x%2, when j == 2c + x//2
            tball = singles.tile([32, NG * 16], F32)
            ident32 = singles.tile([32, 32], F32)
            make_identity(nc, ident32[:])

            # Software pipeline: stage A of group g (S-matmuls, exp, Z, r)
            # is emitted before stage B of group g-1 (c-matmuls, transpose,
            # t) so the PE always has S work queued while the c-matmuls
            # wait on the DVE reciprocal.
            prev = None
            for g in range(NG + 1):
                cur = None
                if g < NG:
                    szg = szp.tile([128, 640], BF16, tag="sz")
                    nc.sync.dma_start(szg[:], szg_t.ap()[g])
                    u = usb.tile([128, 2048], BF16, tag="u")
                    rr = gsb.tile([128, 16], BF16, tag="rr")
                    for c in range(4):
                        # S chunk: [128 n, (di, b, m) 512]
                        sp = pst.tile([128, 512], F32, tag="s")
                        nc.tensor.matmul(
                            sp[:],
                            szg[32 * c:32 * c + 16, 512:640],
                            szg[32 * c:32 * c + 16, 0:512],
                            tile_position=(32 * c, 0),
                        )
                        nc.scalar.activation(
                            u[:, 512 * c:512 * (c + 1)], sp[:], AF.Exp
                        )
                    zs = gsb.tile([128, 16], BF16, tag="zs")
                    u3 = u[:].rearrange("p (s m) -> p s m", s=16)
                    with nc.allow_low_precision("softmax Z in bf16"):
                        nc.vector.tensor_reduce(
                            zs[:], u3,
                            mybir.AxisListType.X, mybir.AluOpType.add,
                        )
                        nc.vector.reciprocal(rr[:], zs[:])
                    cur = (g, u, rr)

                if prev is not None:
                    pg, pu, prr = prev
                    cps = pcs.tile([128, 512], F32, tag="c")
                    for c in range(4):
                        # c rows land on the 32c partition stripe; valid
                        # columns are this chunk's quarter, rest is garbage
                        nc.tensor.matmul(
                            cps[32 * c:32 * c + 4, :],
                            prr[:, 4 * c:4 * c + 4],
                            pu[:, 512 * c:512 * (c + 1)],
                            tile_position=(0, 32 * c),
                        )
                    cs = gsb.tile([128, 512], BF16, tag="cs")
                    nc.scalar.activation(cs[:], cps[:], AF.Copy)
                    # XBAR transpose: ct[m, 128q + p] = cs[p, 128q + m]
                    ct = gsb.tile([128, 512], BF16, tag="ct")
                    nc.sync.dma_start_transpose(
                        ct[:].rearrange("p (q f) -> p q f", q=4), cs[:]
                    )

                    # t: [32 (j,k), 512 (q,p)]; valid at p=32(j//2)+x, q=x,
                    # x=2(j%2)+b; garbage cols dropped by the strided copies
                    tp = psm.tile([32, 512], F32, tag="t")
                    nc.tensor.matmul(
                        tp[:],
                        sna[:, pg * 32:(pg + 1) * 32],
                        ct[:],
                    )
                    # valid col for (x, c) is 129x+32c: extract [32, 4] per x
                    tp3 = tp[:].rearrange("p (q f) -> p q f", q=4)
                    tbv = tball[:].rearrange("p (g x c) -> p g x c", g=NG, x=4)
                    for x in range(4):
                        nc.vector.tensor_copy(
                            tbv[:, pg, x, :],
                            tp3[:, x, :].rearrange(
                                "p (c r) -> p c r", c=4)[:, :, x],
                        )
                prev = cur

            # ---- row-gather t entries via selection matmuls:
            # tb4ps[k, (j, g, b)] = tball[4j+k, 16g + 8*(j%2) + 4b + j//2]
            tb4ps = pt4.tile([4, 1024], F32, tag="t4")
            tbx = tball[:].rearrange("p (g x2 b c) -> p g x2 b c", g=NG, x2=2, b=2)
            for j in range(8):
                nc.tensor.matmul(
                    tb4ps[:, j * 128:(j + 1) * 128],
                    ident32[:, 4 * j:4 * j + 4],
                    tbx[:, :, j % 2, :, j // 2],
                )
            # reorder (j, g, b) -> (g, j, b): tb4 [4, 2*BC]
            tb4 = singles.tile([4, 2 * BC], F32)
            nc.vector.tensor_copy(
                tb4[:].rearrange("k (g j b) -> k j g b", g=NG, j=8),
                tb4ps[:].rearrange("k (j g b) -> k j g b", g=NG, j=8),
            )

            # ---- batched MLP over all BC items
            tb3 = tb4[:].rearrange("p (b j) -> p j b", j=2)
            ps_h = pst.tile([64, BC], F32, tag="s")
            nc.tensor.matmul(ps_h[:], wcrs[:], tb3[:, 0, :], start=True, stop=False)
            nc.tensor.matmul(ps_h[:], wctg[:], tb3[:, 1, :], start=False, stop=True)
            h_sb = singles.tile([64, BC], F32)
            nc.vector.tensor_copy(h_sb[:], ps_h[:])

            ps_z1 = pst.tile([128, BC], F32, tag="s")
            nc.tensor.matmul(ps_z1[:], w1[:], h_sb[:])
            h1 = singles.tile([128, BC], F32)
            nc.scalar.activation(h1[:], ps_z1[:], AF.Tanh, bias=b1[:])

            ps_z2 = pst.tile([128, BC], F32, tag="s")
            nc.tensor.matmul(ps_z2[:], w2[:], h1[:])
            h2 = singles.tile([128, BC], F32)
            nc.scalar.activation(h2[:], ps_z2[:], AF.Tanh, bias=b2[:])

            ps_z3 = psm.tile([1, BC], F32, tag="t")
            nc.tensor.matmul(ps_z3[:], w3[:], h2[:])
            y_sb = singles.tile([1, BC], F32)
            nc.vector.tensor_add(y_sb[:], ps_z3[:], b3r[:])

            nc.sync.dma_start(out_t.ap().rearrange("b o -> o b"), y_sb[:])

    nc.compile()
    return nc


def _host_prep(inputs):
    f = lambda x: np.asarray(x, dtype=np.float32)
    s_obs = f(inputs["s_obs"])

    def aug(W, b):
        return np.vstack([f(W), f(b).reshape(1, -1)])  # [4, dout]

    Wq_rs = aug(inputs["Wq_rs"], inputs["bq_rs"])
    Wk_rs = aug(inputs["Wk_rs"], inputs["bk_rs"])
    Wv_rs = aug(inputs["Wv_rs"], inputs["bv_rs"])
    Wq_tg = aug(inputs["Wq_tg"], inputs["bq_tg"])
    Wk_tg = aug(inputs["Wk_tg"], inputs["bk_tg"])
    Wv_tg = aug(inputs["Wv_tg"], inputs["bv_tg"])

    scale = 1.0 / np.sqrt(16.0)
    A_rs = (Wq_rs @ Wk_rs.T * scale).astype(np.float32)  # [4, 4]
    A_tg = (Wq_tg @ Wk_tg.T * scale).astype(np.float32)

    wcrs = np.zeros((4, 64), np.float32)
    wctg = np.zeros((4, 64), np.float32)
    wcrs[:, 0:32] = Wv_rs / N
    wctg[:, 32:64] = Wv_tg / N

    w1 = f(inputs["W1"])
    b1 = f(inputs["b1"]).reshape(128, 1)
    w2 = f(inputs["W2"])
    b2 = f(inputs["b2"]).reshape(128, 1)
    w3 = f(inputs["W3"])
    b3rep = np.full((1, BC), float(np.asarray(inputs["b3"]).reshape(-1)[0]),
                    np.float32)

    common = dict(wcrs=wcrs, wctg=wctg, w1=w1, w2=w2, w3=w3,
                  b1=b1, b2=b2, b3rep=b3rep)
    in_maps = []
    for core in range(N_CORES):
        sc = s_obs[core * BC:(core + 1) * BC]               # [512, 128, 3]
        sa = np.concatenate(
            [sc, np.ones((BC, N, 1), np.float32)], axis=2)  # [512, 128, 4]
        sa_bf = sa.astype(ml_dtypes.bfloat16)

        # sn [128, 512*4]: [n, 4i+k] = s'_i[n, k]
        sn = np.ascontiguousarray(
            sa_bf.transpose(1, 0, 2).reshape(128, BC * 4))

        # szg [NG, 128, 640]:
        #   cols 0..511:  [32c+8di+4b+k', 256di+128b+m] = s'_{8g+2c+di}[m,k']
        #   cols 512..639: [32c+8di+4b+k', 512+n] = P_{8g+2c+di, b}[n, k']
        szg = np.zeros((NG, 128, 640), ml_dtypes.bfloat16)
        blk = sa_bf.transpose(0, 2, 1).reshape(NG, 8, 4, 128)  # [g, j, k, m]
        Ps = [
            np.einsum('ink,kq->inq', sa, A).astype(ml_dtypes.bfloat16)
            .reshape(NG, 8, 128, 4)
            for A in (A_rs, A_tg)
        ]
        for j in range(8):
            r0 = 32 * (j // 2) + 8 * (j % 2)
            c0 = 256 * (j % 2)
            for b in range(2):
                szg[:, r0 + 4 * b:r0 + 4 * b + 4,
                    c0 + 128 * b:c0 + 128 * b + 128] = blk[:, j]
                szg[:, r0 + 4 * b:r0 + 4 * b + 4, 512:640] = \
                    Ps[b][:, j].transpose(0, 2, 1)

        m = dict(common)
        m["szg"] = szg
        m["sn"] = sn
        in_maps.append(m)
    return in_maps


def kernel(**inputs):
    if "nc" not in _cache:
        _cache["nc"] = _build()
    nc = _cache["nc"]
    in_maps = _host_prep(inputs)
    trace = os.environ.get("KERNEL_TRACE", "0") == "1"
    res = bass_utils.run_bass_kernel_spmd(
        nc, in_maps, core_ids=list(range(N_CORES)), trace=trace
    )
    _cache["last"] = res
    out = np.concatenate([r["out"] for r in res.results], axis=0)
    return out.astype(np.float32)


# revision 22
# speedup vs baseline: 5.1348x; 1.0408x over previous
"""Trainium2 Bass kernel for nn_CriticUAVob (attention-pool critic).

Math per item (4096 total), per branch: S = s' A s'^T (s' = [s|1] [N,4],
A = Wq' Wk'^T / sqrt(dk)), U = exp(S), Z_n = sum_m U[n,m], pooled =
Wv'^T t / N with t_k = sum_m s'[m,k] c_m, c_m = sum_n U[n,m] / Z_n.

Layout strategy (n on partitions): 8 items per group flow through
block-diagonal "superposition" matmuls so the PE runs few, wide bf16
instructions instead of thousands of tiny fp32 ones:
  S    = P^T-chunk^T @ SZ-chunk     (4 matmuls / group, moving 512 bf16)
  U    = exp(S)                     (Scalar engine, PSUM->SBUF bf16)
  Z    = segmented row-sum of U     (DVE tensor_reduce axis=X)
  r    = 1/Z                        (DVE reciprocal)
  c    = r^T @ U                    (4 matmuls into one PSUM tile at
                                     32-aligned partition stripes)
  ct   = transpose(c)               (one XBAR DMA transpose, 4 quarters)
  t    = s_nat^T @ ct               (1 matmul; off-diagonal garbage
                                     columns are ignored at extraction)
P^T (= A^T s'^T) and the zero-padded block-diagonal SZ operand are
precomputed on the host and arrive as one 1280B-per-partition DMA per
group. Final tiny MLP is batched over all 512 items.

Sharding: pure data parallel, batch split across 8 NeuronCores.
"""
import os
import sys
import numpy as np

sys.path.insert(0, "/opt/trn_rl_repo")

import ml_dtypes
import concourse.bass as bass
import concourse.tile as tile
from concourse import bacc, mybir
from concourse import bass_utils
from concourse.masks import make_identity

N_CORES = 8
B = 4096
N = 128
BC = B // N_CORES          # 512 items per core
IPG = 8                    # items per group
NG = BC // IPG             # 64 groups
F32 = mybir.dt.float32
BF16 = mybir.dt.bfloat16
AF = mybir.ActivationFunctionType

_cache = {}


def _build():
    nc = bacc.Bacc(
        "TRN2",
        target_bir_lowering=False,
        debug=False,
        enable_asserts=False,
        num_devices=N_CORES,
    )
    szg_t = nc.dram_tensor("szg", [NG, 128, 640], BF16, kind="ExternalInput")
    sn_t = nc.dram_tensor("sn", [128, BC * 4], BF16, kind="ExternalInput")
    wcrs_t = nc.dram_tensor("wcrs", [4, 64], F32, kind="ExternalInput")
    wctg_t = nc.dram_tensor("wctg", [4, 64], F32, kind="ExternalInput")
    w1_t = nc.dram_tensor("w1", [64, 128], F32, kind="ExternalInput")
    w2_t = nc.dram_tensor("w2", [128, 128], F32, kind="ExternalInput")
    w3_t = nc.dram_tensor("w3", [128, 1], F32, kind="ExternalInput")
    b1_t = nc.dram_tensor("b1", [128, 1], F32, kind="ExternalInput")
    b2_t = nc.dram_tensor("b2", [128, 1], F32, kind="ExternalInput")
    b3_t = nc.dram_tensor("b3rep", [1, BC], F32, kind="ExternalInput")
    out_t = nc.dram_tensor("out", [BC, 1], F32, kind="ExternalOutput")

    with tile.TileContext(nc) as tc:
        with (
            tc.tile_pool(name="singles", bufs=1) as singles,
            tc.tile_pool(name="szp", bufs=4) as szp,
            tc.tile_pool(name="usb", bufs=3) as usb,
            tc.tile_pool(name="gsb", bufs=3) as gsb,
            tc.tile_pool(name="pst", bufs=2, space="PSUM") as pst,
            tc.tile_pool(name="pcs", bufs=2, space="PSUM") as pcs,
            tc.tile_pool(name="psm", bufs=2, space="PSUM") as psm,
        ):
            sna = singles.tile([128, BC * 4], BF16)
            nc.sync.dma_start(sna[:], sn_t.ap())
            wcrs = singles.tile([4, 64], F32)
            nc.sync.dma_start(wcrs[:], wcrs_t.ap())
            wctg = singles.tile([4, 64], F32)
            nc.sync.dma_start(wctg[:], wctg_t.ap())
            w1 = singles.tile([64, 128], F32)
            nc.sync.dma_start(w1[:], w1_t.ap())
            w2 = singles.tile([128, 128], F32)
            nc.sync.dma_start(w2[:], w2_t.ap())
            w3 = singles.tile([128, 1], F32)
            nc.sync.dma_start(w3[:], w3_t.ap())
            b1 = singles.tile([128, 1], F32)
            nc.sync.dma_start(b1[:], b1_t.ap())
            b2 = singles.tile([128, 1], F32)
            nc.sync.dma_start(b2[:], b2_t.ap())
            b3r = singles.tile([1, BC], F32)
            nc.sync.dma_start(b3r[:], b3_t.ap())
            # t accumulator [32, (g, x, c)]: [4j+k, 16g+4x+c] holds t of
            # item 8g+2c+(x//2), branch x%2, when j == 2c + x//2
            tball = singles.tile([32, NG * 16], F32)
            ident32 = singles.tile([32, 32], F32)
            make_identity(nc, ident32[:])

            # Software pipeline: stage A of group g (S-matmuls, exp, Z, r)
            # runs 2 groups ahead of stage B (c-matmuls, transpose, t) so
            # the PE always has S work queued while the c-matmuls wait on
            # the DVE reciprocal and the XBAR transpose round-trips.
            LAG = 2
            inflight = []
            for g in range(NG + LAG):
                if g < NG:
                    szg = szp.tile([128, 640], BF16, tag="sz")
                    nc.sync.dma_start(szg[:], szg_t.ap()[g])
                    u = usb.tile([128, 2048], BF16, tag="u")
                    rr = gsb.tile([128, 16], BF16, tag="rr")
                    for h in range(2):
                        # S chunks: [128 n, (di, b, m) 512] x2 per psum tile
                        sp = pst.tile([128, 1024], F32, tag="s")
                        for cc in range(2):
                            c = 2 * h + cc
                            nc.tensor.matmul(
                                sp[:, 512 * cc:512 * (cc + 1)],
                                szg[32 * c:32 * c + 16, 512:640],
                                szg[32 * c:32 * c + 16, 0:512],
                                tile_position=(32 * c, 0),
                            )
                        nc.scalar.activation(
                            u[:, 1024 * h:1024 * (h + 1)], sp[:], AF.Exp
                        )
                    zs = gsb.tile([128, 16], BF16, tag="zs")
                    u3 = u[:].rearrange("p (s m) -> p s m", s=16)
                    with nc.allow_low_precision("softmax Z in bf16"):
                        nc.vector.tensor_reduce(
                            zs[:], u3,
                            mybir.AxisListType.X, mybir.AluOpType.add,
                        )
                        nc.vector.reciprocal(rr[:], zs[:])
                    inflight.append((g, u, rr))

                if len(inflight) > (LAG if g < NG else 0):
                    pg, pu, prr = inflight.pop(0)
                    cps = pcs.tile([128, 512], F32, tag="c")
                    for c in range(4):
                        # c rows land on the 32c partition stripe; valid
                        # columns are this chunk's quarter, rest is garbage
                        nc.tensor.matmul(
                            cps[32 * c:32 * c + 4, :],
                            prr[:, 4 * c:4 * c + 4],
                            pu[:, 512 * c:512 * (c + 1)],
                            tile_position=(0, 32 * c),
                        )
                    cs = gsb.tile([128, 512], BF16, tag="cs")
                    nc.scalar.activation(cs[:], cps[:], AF.Copy)
                    # XBAR transpose: ct[m, 128q + p] = cs[p, 128q + m]
                    ct = gsb.tile([128, 512], BF16, tag="ct")
                    nc.sync.dma_start_transpose(
                        ct[:].rearrange("p (q f) -> p q f", q=4), cs[:]
                    )

                    # t: [32 (j,k), 512 (q,p)]; valid at p=32(j//2)+x, q=x,
                    # x=2(j%2)+b; garbage cols dropped by the strided copies
                    tp = psm.tile([32, 512], F32, tag="t")
                    nc.tensor.matmul(
                        tp[:],
                        sna[:, pg * 32:(pg + 1) * 32],
                        ct[:],
                    )
                    # valid col for (x, c) is 129x+32c: extract [32, 4] per x
                    tp3 = tp[:].rearrange("p (q f) -> p q f", q=4)
                    tbv = tball[:].rearrange("p (g x c) -> p g x c", g=NG, x=4)
                    for x in range(4):
                        nc.vector.tensor_copy(
                            tbv[:, pg, x, :],
                            tp3[:, x, :].rearrange(
                                "p (c r) -> p c r", c=4)[:, :, x],
                        )

            # ---- row-gather t entries via selection matmuls:
            # tb4ps[k, (j, g, b)] = tball[4j+k, 16g + 8*(j%2) + 4b + j//2]
            tb4 = singles.tile([4, 2 * BC], F32)
            tbx = tball[:].rearrange("p (g x2 b c) -> p g x2 b c", g=NG, x2=2, b=2)
            for half in range(2):
                tb4ps = psm.tile([4, 512], F32, tag="t")
                for jj in range(4):
                    j = 4 * half + jj
                    nc.tensor.matmul(
                        tb4ps[:, jj * 128:(jj + 1) * 128],
                        ident32[:, 4 * j:4 * j + 4],
                        tbx[:, :, j % 2, :, j // 2],
                    )
                # reorder (j, g, b) -> (g, j, b) within this half
                nc.vector.tensor_copy(
                    tb4[:].rearrange(
                        "k (g jh j b) -> k jh j g b", g=NG, jh=2, j=4
                    )[:, half],
                    tb4ps[:].rearrange("k (j g b) -> k j g b", g=NG, j=4),
                )

            # ---- batched MLP over all BC items
            tb3 = tb4[:].rearrange("p (b j) -> p j b", j=2)
            ps_h = pst.tile([64, BC], F32, tag="s")
            nc.tensor.matmul(ps_h[:], wcrs[:], tb3[:, 0, :], start=True, stop=False)
            nc.tensor.matmul(ps_h[:], wctg[:], tb3[:, 1, :], start=False, stop=True)
            h_sb = singles.tile([64, BC], F32)
            nc.vector.tensor_copy(h_sb[:], ps_h[:])

            ps_z1 = pst.tile([128, BC], F32, tag="s")
            nc.tensor.matmul(ps_z1[:], w1[:], h_sb[:])
            h1 = singles.tile([128, BC], F32)
            nc.scalar.activation(h1[:], ps_z1[:], AF.Tanh, bias=b1[:])

            ps_z2 = pst.tile([128, BC], F32, tag="s")
            nc.tensor.matmul(ps_z2[:], w2[:], h1[:])
            h2 = singles.tile([128, BC], F32)
            nc.scalar.activation(h2[:], ps_z2[:], AF.Tanh, bias=b2[:])

            ps_z3 = psm.tile([1, BC], F32, tag="t")
            nc.tensor.matmul(ps_z3[:], w3[:], h2[:])
            y_sb = singles.tile([1, BC], F32)
            nc.vector.tensor_add(y_sb[:], ps_z3[:], b3r[:])

            nc.sync.dma_start(out_t.ap().rearrange("b o -> o b"), y_sb[:])

    nc.compile()
    return nc


def _host_prep(inputs):
    f = lambda x: np.asarray(x, dtype=np.float32)
    s_obs = f(inputs["s_obs"])

    def aug(W, b):
        return np.vstack([f(W), f(b).reshape(1, -1)])  # [4, dout]

    Wq_rs = aug(inputs["Wq_rs"], inputs["bq_rs"])
    Wk_rs = aug(inputs["Wk_rs"], inputs["bk_rs"])
    Wv_rs = aug(inputs["Wv_rs"], inputs["bv_rs"])
    Wq_tg = aug(inputs["Wq_tg"], inputs["bq_tg"])
    Wk_tg = aug(inputs["Wk_tg"], inputs["bk_tg"])
    Wv_tg = aug(inputs["Wv_tg"], inputs["bv_tg"])

    scale = 1.0 / np.sqrt(16.0)
    A_rs = (Wq_rs @ Wk_rs.T * scale).astype(np.float32)  # [4, 4]
    A_tg = (Wq_tg @ Wk_tg.T * scale).astype(np.float32)

    wcrs = np.zeros((4, 64), np.float32)
    wctg = np.zeros((4, 64), np.float32)
    wcrs[:, 0:32] = Wv_rs / N
    wctg[:, 32:64] = Wv_tg / N

    w1 = f(inputs["W1"])
    b1 = f(inputs["b1"]).reshape(128, 1)
    w2 = f(inputs["W2"])
    b2 = f(inputs["b2"]).reshape(128, 1)
    w3 = f(inputs["W3"])
    b3rep = np.full((1, BC), float(np.asarray(inputs["b3"]).reshape(-1)[0]),
                    np.float32)

    common = dict(wcrs=wcrs, wctg=wctg, w1=w1, w2=w2, w3=w3,
                  b1=b1, b2=b2, b3rep=b3rep)
    in_maps = []
    for core in range(N_CORES):
        sc = s_obs[core * BC:(core + 1) * BC]               # [512, 128, 3]
        sa = np.concatenate(
            [sc, np.ones((BC, N, 1), np.float32)], axis=2)  # [512, 128, 4]
        sa_bf = sa.astype(ml_dtypes.bfloat16)

        # sn [128, 512*4]: [n, 4i+k] = s'_i[n, k]
        sn = np.ascontiguousarray(
            sa_bf.transpose(1, 0, 2).reshape(128, BC * 4))

        # szg [NG, 128, 640]:
        #   cols 0..511:  [32c+8di+4b+k', 256di+128b+m] = s'_{8g+2c+di}[m,k']
        #   cols 512..639: [32c+8di+4b+k', 512+n] = P_{8g+2c+di, b}[n, k']
        szg = np.zeros((NG, 128, 640), ml_dtypes.bfloat16)
        blk = sa_bf.transpose(0, 2, 1).reshape(NG, 8, 4, 128)  # [g, j, k, m]
        Ps = [
            np.einsum('ink,kq->inq', sa, A).astype(ml_dtypes.bfloat16)
            .reshape(NG, 8, 128, 4)
            for A in (A_rs, A_tg)
        ]
        for j in range(8):
            r0 = 32 * (j // 2) + 8 * (j % 2)
            c0 = 256 * (j % 2)
            for b in range(2):
                szg[:, r0 + 4 * b:r0 + 4 * b + 4,
                    c0 + 128 * b:c0 + 128 * b + 128] = blk[:, j]
                szg[:, r0 + 4 * b:r0 + 4 * b + 4, 512:640] = \
                    Ps[b][:, j].transpose(0, 2, 1)

        m = dict(common)
        m["szg"] = szg
        m["sn"] = sn
        in_maps.append(m)
    return in_maps


def kernel(**inputs):
    if "nc" not in _cache:
        _cache["nc"] = _build()
    nc = _cache["nc"]
    in_maps = _host_prep(inputs)
    trace = os.environ.get("KERNEL_TRACE", "0") == "1"
    res = bass_utils.run_bass_kernel_spmd(
        nc, in_maps, core_ids=list(range(N_CORES)), trace=trace
    )
    _cache["last"] = res
    out = np.concatenate([r["out"] for r in res.results], axis=0)
    return out.astype(np.float32)
